# revision 39
# baseline (speedup 1.0000x reference)
"""Llama GQA attention (B=2, S=2048, H=4096, 32 q heads / 8 kv heads, HD=128)
on 8 Trainium2 NeuronCores.

Sharding: DP=2 over batch x TP=4 over heads.
  core c: batch b = c // 4, tp rank r = c % 4
  - owns q heads [8r, 8r+8), kv heads [2r, 2r+2)
  - computes attention for its heads over its batch
  - o_proj: LOCAL partial over its 1024 attn features for ALL 4096 out
    cols (wo sharded by ROWS), then bf16 ReduceScatter(add) within each
    4-core batch group -> each core holds out cols [1024r, 1024(r+1)).

All inputs are pre-cast to bf16 on the host (weights in panel-major
layouts so every DMA is contiguous), so the device pipeline has zero
dtype-conversion work and no DRAM scratch round-trips.

On-chip layout is fully "transposed" ([feature, token]):
  QT/KT: [d, t] (weight tiles stationary, X^T moving)
  V:     [t, d] (X^T tiles stationary, wv moving)
  S^T[k, q] = (KT tile).T @ QT          (contraction d on partitions)
  P^T = exp(scale * S^T)                (ScalarE, fp32 PSUM -> bf16 SBUF)
  attn^T[d, q] += (V tile).T @ P^T      (contraction k-tokens on partitions)
  out[t, oc]  += (attnb tile).T @ wo    (contraction d on partitions)
Causal masking: only lower-triangle k-tiles are computed; diagonal
128x512 tiles use one of 4 static 0/1 masks (multiplied into P^T on DVE).
Softmax skips max-subtraction (scores are O(7), exp fits fp32).

Denominators: P^T tiles are accumulated on the Pool engine into a
per-head running sum (bf16), then ONE ones-matmul per (head, chunk)
produces the denominator — instead of one matmul per k-tile.

Attention processes heads in PAIRS sharing the same kv head, software-
pipelined so the exp->mask->pa chain of one head hides under the other
head's matmuls:
  per kt: [score(h0), score(h1), pa(h0, kt-1), pa(h1, kt-1)]
X^T for chunk c+1 is DMA'd (pure bf16 copy, no engine work) into a
2-slot rotation at the start of proj(c).
"""

import os
import sys

for _p in ("/opt/trn_rl_repo",):
    if _p not in sys.path:
        sys.path.append(_p)

import numpy as np
import ml_dtypes

import concourse.bacc as bacc
import concourse.mybir as mybir
import concourse.tile as tile
from concourse.bass_utils import run_bass_kernel_spmd

F32 = mybir.dt.float32
BF16 = mybir.dt.bfloat16
B16NP = ml_dtypes.bfloat16

B, S, H = 2, 2048, 4096
NH, NKV, HD = 32, 8, 128
N_CORES = 8
TP = 4
GROUPS = [[0, 1, 2, 3], [4, 5, 6, 7]]

HL = NH // TP          # 8 local q heads
KVL = NKV // TP        # 2 local kv heads
QCOLS = HL * HD        # 1024 local q cols
KVCOLS = KVL * HD      # 256 local kv cols
OC = H // TP           # 1024 out cols per core after ReduceScatter

TC = 512               # token chunk (= one attention q-block)
NCHUNK = S // TC       # 4
KT = H // 128          # 32 contraction tiles for the projections
SCALE = float(HD ** -0.5)

LAST_RESULT = None
_BUILT = {}

EXP = mybir.ActivationFunctionType.Exp
MUL = mybir.AluOpType.mult
ADD = mybir.AluOpType.add


def _build():
    nc = bacc.Bacc("TRN2", debug=False, num_devices=N_CORES)

    xt_d = nc.dram_tensor("xt", [H, S], BF16, kind="ExternalInput").ap()
    cos_d = nc.dram_tensor("cos_t", [HD, S], BF16, kind="ExternalInput").ap()
    sin_d = nc.dram_tensor("sin_t", [HD, S], BF16, kind="ExternalInput").ap()
    wq_d = nc.dram_tensor("wqp", [HL, 128, KT * 128], BF16, kind="ExternalInput").ap()
    wk_d = nc.dram_tensor("wkp", [KVL, 128, KT * 128], BF16, kind="ExternalInput").ap()
    wv_d = nc.dram_tensor("wvp", [128, KT * KVCOLS], BF16, kind="ExternalInput").ap()
    wo_d = nc.dram_tensor("wop", [HL, 128, H], BF16, kind="ExternalInput").ap()
    mask_d = nc.dram_tensor("masks", [HD, 8 * TC], BF16, kind="ExternalInput").ap()
    ones_d = nc.dram_tensor("onesb", [128, 128], BF16, kind="ExternalInput").ap()
    out_d = nc.dram_tensor("out_t", [S, OC], BF16, kind="ExternalOutput").ap()

    with tile.TileContext(nc) as tc:
        with tc.tile_pool(name="sb", bufs=1) as sb, \
             tc.tile_pool(name="ps", bufs=1, space="PSUM") as ps, \
             tc.tile_pool(name="dr", bufs=1, space="DRAM") as dr:

            # ---- persistent tiles ----
            cos_sb = sb.tile([HD, S], BF16)
            sin_sb = sb.tile([HD, S], BF16)
            mask_sb = sb.tile([HD, 8 * TC], BF16)
            ones_sb = sb.tile([128, 128], BF16)
            ktb = sb.tile([128, KVL * S], BF16)             # roped K^T
            vb = sb.tile([128, (S // 128) * KVCOLS], BF16)  # V, [t, tt*256 + d]
            # X^T bf16, 2-chunk rotation: [128, kt*(2*TC) + slot*TC + t]
            xtb = sb.tile([128, KT * 2 * TC], BF16)

            _WB_PRE = {}

            def prefetch_panel(h):
                wb = sb.tile([128, KT * 128], BF16, tag="wb", bufs=3,
                             name="wb")
                nc.sync.dma_start(wb[:], wq_d[h])
                _WB_PRE[h] = wb
                return wb

            # first two q panels dispatched before anything else so the
            # very first matmuls are not stuck behind other transfers
            prefetch_panel(0)
            prefetch_panel(1)
            prefetch_panel(2)
            nc.sync.dma_start(cos_sb[:], cos_d[:])
            nc.sync.dma_start(sin_sb[:], sin_d[:])
            nc.sync.dma_start(mask_sb[:], mask_d[:])
            nc.sync.dma_start(ones_sb[:], ones_d[:])
            # V weights are small enough to keep resident: ONE load, reused
            # by every chunk's V projection.
            wvb = sb.tile([128, KT * KVCOLS], BF16, name="wvb")
            nc.sync.dma_start(wvb[:], wv_d[:])

            def xt_slot(c):
                return c % 2

            def load_xt(c):
                """Four batched DMAs (8 kt-tiles each) bringing chunk c of
                X^T into its rotation slot. (Each DMA instruction costs
                ~0.7us of serial dispatch on the Sync sequencer — batch;
                but keep kt-granularity coarse enough for the first
                matmuls to start before the whole chunk lands.)"""
                sl = xt_slot(c)
                dst = xtb.rearrange("p (kt s t) -> p kt s t", s=2, t=TC)[:, :, sl, :]
                src = xt_d[:, c * TC:(c + 1) * TC].rearrange(
                    "(kt p) t -> p kt t", p=128)
                for g in range(4):
                    nc.sync.dma_start(dst[:, g * 8:(g + 1) * 8, :],
                                      src[:, g * 8:(g + 1) * 8, :])

            def xt_tile(c, kt, lo=0, width=TC):
                sl = xt_slot(c)
                base = kt * 2 * TC + sl * TC + lo
                return xtb[:, base:base + width]

            _ROPE = {"qf": None, "n": 0, "t0": 0, "dst": None}

            def rope(dst, pq, t0):
                """Batch TWO heads per rope: psum drains on DVE (keeps the
                Scalar engine free so attention exps start immediately),
                half-swap DMAs on the idle Scalar DGE, fused DVE math."""
                st = _ROPE
                if st["qf"] is None:
                    qf = sb.tile([128, 2 * TC], F32, tag="qf", bufs=1)
                    nc.vector.tensor_copy(qf[:, :TC], pq[:])
                    st.update(qf=qf, n=1, t0=t0, dst=dst)
                    return
                qf = st["qf"]
                nc.vector.tensor_copy(qf[:, TC:], pq[:])
                qs = sb.tile([128, 2 * TC], F32, tag="qs", bufs=1)
                nc.scalar.dma_start(qs[0:64, :], qf[64:128, :])
                nc.scalar.dma_start(qs[64:128, :], qf[0:64, :])
                cseg = cos_sb[:, t0:t0 + TC]
                sseg = sin_sb[:, t0:t0 + TC]
                nc.vector.tensor_tensor(qf[:, :TC], qf[:, :TC], cseg, MUL)
                nc.vector.tensor_tensor(qf[:, TC:], qf[:, TC:], cseg, MUL)
                nc.vector.tensor_tensor(qs[:, :TC], qs[:, :TC], sseg, MUL)
                nc.vector.tensor_tensor(qs[:, TC:], qs[:, TC:], sseg, MUL)
                dst0 = st["dst"]
                nc.vector.tensor_tensor(dst0, qf[:, :TC], qs[:, :TC], ADD)
                nc.vector.tensor_tensor(dst, qf[:, TC:], qs[:, TC:], ADD)
                st.update(qf=None, n=0, dst=None)

            qtb = None

            def proj(c):
                nonlocal qtb
                t0 = c * TC
                qtb = sb.tile([128, HL * TC], BF16, tag="qt", bufs=1, name="qtb")
                for h in range(HL):
                    wb = _WB_PRE.pop(h, None) if c == 0 else None
                    if wb is None:
                        wb = sb.tile([128, KT * 128], BF16, tag="wb", bufs=3,
                                     name="wb")
                        nc.sync.dma_start(wb[:], wq_d[h])
                    pq = ps.tile([128, TC], F32, tag="pj", bufs=2, name="pq")
                    for kt in range(KT):
                        nc.tensor.matmul(
                            pq[:], wb[:, kt * 128:(kt + 1) * 128],
                            xt_tile(c, kt),
                            start=(kt == 0), stop=(kt == KT - 1))
                    rope(qtb[:, h * TC:(h + 1) * TC], pq, t0)
                    if h == 2 and c + 1 < NCHUNK:
                        # next chunk's X^T, after the first panels are in
                        # flight so it doesn't delay them in dispatch order
                        load_xt(c + 1)
                for kv in range(KVL):
                    wb = sb.tile([128, KT * 128], BF16, tag="wb", bufs=3, name="wbk")
                    nc.sync.dma_start(wb[:], wk_d[kv])
                    pk = ps.tile([128, TC], F32, tag="pj", bufs=2, name="pk")
                    for kt in range(KT):
                        nc.tensor.matmul(
                            pk[:], wb[:, kt * 128:(kt + 1) * 128],
                            xt_tile(c, kt),
                            start=(kt == 0), stop=(kt == KT - 1))
                    rope(ktb[:, kv * S + t0:kv * S + t0 + TC], pk, t0)
                # V: lhsT = X^T tiles (stationary), rhs = wv (both kv heads
                # at once, 256-wide) -> V[t, c] accumulated over kt.
                for tt in range(TC // 128):
                    pv = ps.tile([128, KVCOLS], F32, tag="pj", bufs=2, name="pv")
                    for kt in range(KT):
                        lx = xt_tile(c, kt, tt * 128, 128)
                        nc.tensor.matmul(
                            pv[:], lx,
                            wvb[:, kt * KVCOLS:(kt + 1) * KVCOLS],
                            start=(kt == 0), stop=(kt == KT - 1))
                    vt_idx = (t0 // 128) + tt
                    nc.scalar.copy(
                        vb[:, vt_idx * KVCOLS:(vt_idx + 1) * KVCOLS], pv[:])

            attnb = None
            _WOR_PRE = {}

            def prefetch_wor(ocg):
                """ONE batched DMA for the 8 wo rhs slices of output group
                ocg: tile [128, ft*TC + oc]."""
                w = sb.tile([128, HL * TC], BF16, tag="wor", bufs=2,
                            name="wor")
                nc.sync.dma_start(
                    w.rearrange("p (f c) -> p f c", c=TC),
                    wo_d.rearrange("f p c -> p f c")[:, :, ocg * TC:(ocg + 1) * TC])
                _WOR_PRE[ocg] = w
                return w

            def attention(c):
                """Heads processed in pairs sharing a kv head; adjacent in
                qtb, so one [128, 2*TC] score matmul + one fused exp serve
                both. pa/pd accumulate per head; masks alternate DVE/GpSimd."""
                nonlocal attnb
                nkt = 4 * c + 4
                attnb = sb.tile([128, HL * TC], BF16, tag="attn", bufs=1,
                                name="attnb")
                # For chunk 0 (every k-tile diagonal, heavy DVE masking) the
                # denominator rides per-k-tile ones-matmuls on TensorE. For
                # chunks 1-3 the P^T tiles are instead accumulated into
                # per-head running sums (h0-half on DVE, h1-half on GpSimd,
                # both of which have slack) and the denominator costs ONE
                # matmul per head — removing a third of attention's
                # TensorE work.
                use_ptsum = (c > 0)
                for p in range(HL // 2):
                    h0, h1 = 2 * p, 2 * p + 1
                    kv = h0 // (HL // KVL)
                    qpair = qtb[:, h0 * TC:(h0 + 2) * TC]
                    pa0 = ps.tile([128, TC], F32, tag="pa", bufs=2, name="pa0")
                    pa1 = ps.tile([128, TC], F32, tag="pa", bufs=2, name="pa1")
                    pd0 = ps.tile([128, TC], F32, tag="pj", bufs=2, name="pd0")
                    pd1 = ps.tile([128, TC], F32, tag="pj", bufs=2, name="pd1")
                    if use_ptsum:
                        ps0 = sb.tile([128, TC], BF16, tag="ptsum", bufs=2,
                                      name="ps0")
                        ps1 = sb.tile([128, TC], BF16, tag="ptsum", bufs=2,
                                      name="ps1")

                    def emit_pa_pd(kt, pt):
                        st, sp = (kt == 0), (kt == nkt - 1)
                        vt = vb[:, kt * KVCOLS + kv * 128:
                                kt * KVCOLS + (kv + 1) * 128]
                        nc.tensor.matmul(pa0[:], vt, pt[:, :TC],
                                         start=st, stop=sp)
                        nc.tensor.matmul(pa1[:], vt, pt[:, TC:],
                                         start=st, stop=sp)
                        if not use_ptsum:
                            nc.tensor.matmul(pd0[:], ones_sb[:], pt[:, :TC],
                                             start=st, stop=sp)
                            nc.tensor.matmul(pd1[:], ones_sb[:], pt[:, TC:],
                                             start=st, stop=sp)

                    pending = []
                    for kt in range(nkt):
                        kts = ktb[:, kv * S + kt * 128:kv * S + (kt + 1) * 128]
                        sps = ps.tile([128, 2 * TC], F32, tag="s", bufs=2,
                                      name="sps")
                        # matmul out must stay within one PSUM bank: write
                        # the fused tile's halves with two 512-wide matmuls
                        # (same stationary K tile), then ONE fused exp.
                        nc.tensor.matmul(sps[:, :TC], kts,
                                         qpair[:, :TC], start=True, stop=True)
                        nc.tensor.matmul(sps[:, TC:], kts,
                                         qpair[:, TC:], start=True, stop=True)
                        # pa/pd run TWO k-tiles behind the scores: the extra
                        # slack hides exp latency AND the previous pair's
                        # normalize chain (which frees the pa psum slots).
                        if len(pending) >= 2:
                            emit_pa_pd(*pending.pop(0))
                        pt = sb.tile([128, 2 * TC], BF16, tag="pt", bufs=5,
                                     name="pt")
                        nc.scalar.activation(pt[:], sps[:], EXP, scale=SCALE)
                        j = kt - 4 * c
                        if j >= 0:
                            # masks on DVE: GpSimd must stay nearly free so
                            # a collective's completion wait blocks little
                            msk = mask_sb[:, j * 2 * TC:(j + 1) * 2 * TC]
                            nc.vector.tensor_tensor(pt[:], pt[:], msk, MUL)
                        if use_ptsum:
                            if kt == 0:
                                nc.vector.tensor_copy(ps0[:], pt[:, :TC])
                                nc.gpsimd.tensor_copy(ps1[:], pt[:, TC:])
                            else:
                                nc.vector.tensor_tensor(
                                    ps0[:], ps0[:], pt[:, :TC], ADD)
                                nc.gpsimd.tensor_tensor(
                                    ps1[:], ps1[:], pt[:, TC:], ADD)
                        pending.append((kt, pt))
                    while pending:
                        emit_pa_pd(*pending.pop(0))
                    if use_ptsum:
                        nc.tensor.matmul(pd0[:], ones_sb[:], ps0[:],
                                         start=True, stop=True)
                        nc.tensor.matmul(pd1[:], ones_sb[:], ps1[:],
                                         start=True, stop=True)
                    # stage pd through SBUF (ACT copy): frees the pd
                    # psum slot early, and the custom-DVE approx reciprocal
                    # reads SBUF (it is unreliable on PSUM inputs)
                    pdf0 = sb.tile([128, TC], F32, tag="pdf", bufs=2, name="pdf0")
                    nc.scalar.copy(pdf0[:], pd0[:])
                    pdf1 = sb.tile([128, TC], F32, tag="pdf", bufs=2, name="pdf1")
                    nc.scalar.copy(pdf1[:], pd1[:])
                    rc0 = sb.tile([128, TC], F32, tag="rc", bufs=1, name="rc0")
                    nc.vector.reciprocal_approx_fast(rc0[:], pdf0[:])
                    nc.vector.tensor_tensor(
                        attnb[:, h0 * TC:(h0 + 1) * TC], pa0[:], rc0[:], MUL)
                    rc1 = sb.tile([128, TC], F32, tag="rc", bufs=1, name="rc1")
                    nc.vector.reciprocal_approx_fast(rc1[:], pdf1[:])
                    nc.vector.tensor_tensor(
                        attnb[:, h1 * TC:(h1 + 1) * TC], pa1[:], rc1[:], MUL)
                    if p == 2:
                        # hoist first o-proj weight loads under attention
                        prefetch_wor(0)

            def outproj(c):
                """Local partial o_proj -> cci[rank, t, oc], then bf16
                ReduceScatter(add) within the 4-core group. The cco->out_t
                DMAs are deferred to the end of the program so no in-order
                DMA queue mid-stream carries a descriptor that has to wait
                for a collective. The last chunk's RS is split in two
                halves (even / odd output groups) so the first RS overlaps
                the second half of the out-projection matmuls."""
                t0 = c * TC
                ccis = [dr.tile([TP, TC, OC // 2], BF16, tag="cci",
                                bufs=4, name=f"cci{h}") for h in range(2)]
                order = [0, 2, 4, 6, 1, 3, 5, 7]

                pending_out = []

                def emit_rs(half):
                    """Trigger the RS for one half of the output columns.
                    The cco->out DMA rides the GpSimd stream (its wait for
                    the RS blocks only GpSimd, which hosts no other work —
                    a sync-engine DMA here would stall ALL DMA dispatch for
                    the collective's latency), and is DEFERRED past any
                    later RS trigger so back-to-back collectives don't
                    serialize on it."""
                    cco = dr.tile([TC, OC // 2], BF16, tag="cco", bufs=4,
                                  name=f"cco{half}")
                    dst = out_d[t0:t0 + TC,
                                half * (OC // 2):(half + 1) * (OC // 2)]
                    nc.gpsimd.collective_compute(
                        "ReduceScatter", ADD,
                        replica_groups=GROUPS, ins=[ccis[half][:]],
                        outs=[cco[:]])
                    pending_out.append((dst, cco))

                for i, ocg in enumerate(order):
                    wor = _WOR_PRE.pop(ocg, None)
                    if wor is None:
                        wor = prefetch_wor(ocg)
                    if i + 1 < 8:
                        prefetch_wor(order[i + 1])
                    cci = ccis[ocg % 2]
                    ot = sb.tile([128, 4 * TC], BF16, tag="ot", bufs=2,
                                 name="ot")
                    for tt in range(TC // 128):
                        po = ps.tile([128, TC], F32, tag="s", bufs=2, name="po")
                        for ft in range(HL):
                            nc.tensor.matmul(
                                po[:],
                                attnb[:, ft * TC + tt * 128:ft * TC + (tt + 1) * 128],
                                wor[:, ft * TC:(ft + 1) * TC],
                                start=(ft == 0), stop=(ft == HL - 1))
                        nc.scalar.copy(ot[:, tt * TC:(tt + 1) * TC], po[:])
                    nc.sync.dma_start(
                        cci[ocg // 2].rearrange("(tt p) oc -> p tt oc", p=128),
                        ot.rearrange("p (tt oc) -> p tt oc", oc=TC))
                    if i == 3:
                        emit_rs(0)
                emit_rs(1)
                for dst, cco in pending_out:
                    nc.gpsimd.dma_start(dst, cco[:])

            # ---- chunk loop ----
            load_xt(0)
            for c in range(NCHUNK):
                proj(c)
                attention(c)
                outproj(c)

    nc.compile()
    return nc


def _get_nc():
    if "nc" not in _BUILT:
        _BUILT["nc"] = _build()
    return _BUILT["nc"]


def kernel(hidden_states, cos, sin, wq, wk, wv, wo):
    global LAST_RESULT
    nc = _get_nc()

    hidden_states = np.asarray(hidden_states, dtype=np.float32)
    cos = np.asarray(cos, dtype=np.float32)
    sin = np.asarray(sin, dtype=np.float32)
    wq = np.asarray(wq, dtype=np.float32)
    wk = np.asarray(wk, dtype=np.float32)
    wv = np.asarray(wv, dtype=np.float32)
    wo = np.asarray(wo, dtype=np.float32)

    # host-side shard prep (bf16, panel-major so every DMA is contiguous)
    xts = [np.ascontiguousarray(hidden_states[b].T).astype(B16NP)
           for b in range(B)]
    cts = [np.ascontiguousarray(cos[b].T).astype(B16NP) for b in range(B)]
    sin_eff = []
    for b in range(B):
        se = np.ascontiguousarray(sin[b].T).copy()
        se[0:64, :] *= -1.0
        sin_eff.append(se.astype(B16NP))

    # fused pair masks: each diagonal mask duplicated for the head pair
    masks = np.zeros((HD, 8 * TC), dtype=B16NP)
    for j in range(4):
        m = ((np.arange(HD)[:, None] + 128 * j)
             <= np.arange(TC)[None, :]).astype(B16NP)
        masks[:, j * 2 * TC:j * 2 * TC + TC] = m
        masks[:, j * 2 * TC + TC:(j + 1) * 2 * TC] = m
    onesb = np.ones((128, 128), dtype=B16NP)

    in_maps = []
    for core in range(N_CORES):
        b, r = core // TP, core % TP
        wq_s = wq[:, r * QCOLS:(r + 1) * QCOLS]
        wk_s = wk[:, r * KVCOLS:(r + 1) * KVCOLS]
        wv_s = wv[:, r * KVCOLS:(r + 1) * KVCOLS]
        wo_s = wo[r * QCOLS:(r + 1) * QCOLS, :]
        wqp = np.ascontiguousarray(
            wq_s.reshape(KT, 128, HL, 128).transpose(2, 1, 0, 3)
            .reshape(HL, 128, KT * 128)).astype(B16NP)
        wkp = np.ascontiguousarray(
            wk_s.reshape(KT, 128, KVL, 128).transpose(2, 1, 0, 3)
            .reshape(KVL, 128, KT * 128)).astype(B16NP)
        wvp = np.ascontiguousarray(
            wv_s.reshape(KT, 128, KVCOLS).transpose(1, 0, 2)
            .reshape(128, KT * KVCOLS)).astype(B16NP)
        wop = np.ascontiguousarray(
            wo_s.reshape(HL, 128, H)).astype(B16NP)
        in_maps.append({
            "xt": xts[b],
            "cos_t": cts[b],
            "sin_t": sin_eff[b],
            "wqp": wqp,
            "wkp": wkp,
            "wvp": wvp,
            "wop": wop,
            "masks": masks,
            "onesb": onesb,
        })

    res = run_bass_kernel_spmd(nc, in_maps, core_ids=list(range(N_CORES)))
    LAST_RESULT = res

    out = np.empty((B, S, H), dtype=np.float32)
    for core in range(N_CORES):
        b, r = core // TP, core % TP
        out[b, :, r * OC:(r + 1) * OC] = res.results[core]["out_t"].astype(
            np.float32)
    return out


# revision 40
# speedup vs baseline: 1.0106x; 1.0106x over previous
"""Llama GQA attention (B=2, S=2048, H=4096, 32 q heads / 8 kv heads, HD=128)
on 8 Trainium2 NeuronCores.

Sharding: DP=2 over batch x TP=4 over heads.
  core c: batch b = c // 4, tp rank r = c % 4
  - owns q heads [8r, 8r+8), kv heads [2r, 2r+2)
  - computes attention for its heads over its batch
  - o_proj: LOCAL partial over its 1024 attn features for ALL 4096 out
    cols (wo sharded by ROWS), then bf16 ReduceScatter(add) within each
    4-core batch group -> each core holds out cols [1024r, 1024(r+1)).

All inputs are pre-cast to bf16 on the host (weights in panel-major
layouts so every DMA is contiguous), so the device pipeline has zero
dtype-conversion work and no DRAM scratch round-trips.

On-chip layout is fully "transposed" ([feature, token]):
  QT/KT: [d, t] (weight tiles stationary, X^T moving)
  V:     [t, d] (X^T tiles stationary, wv moving)
  S^T[k, q] = (KT tile).T @ QT          (contraction d on partitions)
  P^T = exp(scale * S^T)                (ScalarE, fp32 PSUM -> bf16 SBUF)
  attn^T[d, q] += (V tile).T @ P^T      (contraction k-tokens on partitions)
  out[t, oc]  += (attnb tile).T @ wo    (contraction d on partitions)
Causal masking: only lower-triangle k-tiles are computed; diagonal
128x512 tiles use one of 4 static 0/1 masks (multiplied into P^T on DVE).
Softmax skips max-subtraction (scores are O(7), exp fits fp32).

Denominators: P^T tiles are accumulated on the Pool engine into a
per-head running sum (bf16), then ONE ones-matmul per (head, chunk)
produces the denominator — instead of one matmul per k-tile.

Attention processes heads in PAIRS sharing the same kv head, software-
pipelined so the exp->mask->pa chain of one head hides under the other
head's matmuls:
  per kt: [score(h0), score(h1), pa(h0, kt-1), pa(h1, kt-1)]
X^T for chunk c+1 is DMA'd (pure bf16 copy, no engine work) into a
2-slot rotation at the start of proj(c).
"""

import os
import sys

for _p in ("/opt/trn_rl_repo",):
    if _p not in sys.path:
        sys.path.append(_p)

import numpy as np
import ml_dtypes

import concourse.bacc as bacc
import concourse.mybir as mybir
import concourse.tile as tile
from concourse.bass_utils import run_bass_kernel_spmd

F32 = mybir.dt.float32
BF16 = mybir.dt.bfloat16
B16NP = ml_dtypes.bfloat16

B, S, H = 2, 2048, 4096
NH, NKV, HD = 32, 8, 128
N_CORES = 8
TP = 4
GROUPS = [[0, 1, 2, 3], [4, 5, 6, 7]]

HL = NH // TP          # 8 local q heads
KVL = NKV // TP        # 2 local kv heads
QCOLS = HL * HD        # 1024 local q cols
KVCOLS = KVL * HD      # 256 local kv cols
OC = H // TP           # 1024 out cols per core after ReduceScatter

TC = 512               # token chunk (= one attention q-block)
NCHUNK = S // TC       # 4
KT = H // 128          # 32 contraction tiles for the projections
SCALE = float(HD ** -0.5)

LAST_RESULT = None
_BUILT = {}

EXP = mybir.ActivationFunctionType.Exp
MUL = mybir.AluOpType.mult
ADD = mybir.AluOpType.add


def _build():
    nc = bacc.Bacc("TRN2", debug=False, num_devices=N_CORES)

    xt_d = nc.dram_tensor("xt", [H, S], BF16, kind="ExternalInput").ap()
    cos_d = nc.dram_tensor("cos_t", [HD, S], BF16, kind="ExternalInput").ap()
    sin_d = nc.dram_tensor("sin_t", [HD, S], BF16, kind="ExternalInput").ap()
    wq_d = nc.dram_tensor("wqp", [HL, 128, KT * 128], BF16, kind="ExternalInput").ap()
    wk_d = nc.dram_tensor("wkp", [KVL, 128, KT * 128], BF16, kind="ExternalInput").ap()
    wv_d = nc.dram_tensor("wvp", [128, KT * KVCOLS], BF16, kind="ExternalInput").ap()
    wo_d = nc.dram_tensor("wop", [HL, 128, H], BF16, kind="ExternalInput").ap()
    mask_d = nc.dram_tensor("masks", [HD, 8 * TC], BF16, kind="ExternalInput").ap()
    ones_d = nc.dram_tensor("onesb", [128, 128], BF16, kind="ExternalInput").ap()
    out_d = nc.dram_tensor("out_t", [S, OC], BF16, kind="ExternalOutput").ap()

    with tile.TileContext(nc) as tc:
        with tc.tile_pool(name="sb", bufs=1) as sb, \
             tc.tile_pool(name="ps", bufs=1, space="PSUM") as ps, \
             tc.tile_pool(name="dr", bufs=1, space="DRAM") as dr:

            # ---- persistent tiles ----
            cos_sb = sb.tile([HD, S], BF16)
            sin_sb = sb.tile([HD, S], BF16)
            mask_sb = sb.tile([HD, 8 * TC], BF16)
            ones_sb = sb.tile([128, 128], BF16)
            ktb = sb.tile([128, KVL * S], BF16)             # roped K^T
            vb = sb.tile([128, (S // 128) * KVCOLS], BF16)  # V, [t, tt*256 + d]
            # X^T bf16, 2-chunk rotation: [128, kt*(2*TC) + slot*TC + t]
            xtb = sb.tile([128, KT * 2 * TC], BF16)

            _WB_PRE = {}

            def prefetch_panel(h):
                wb = sb.tile([128, KT * 128], BF16, tag="wb", bufs=3,
                             name="wb")
                nc.sync.dma_start(wb[:], wq_d[h])
                _WB_PRE[h] = wb
                return wb

            # first two q panels dispatched before anything else so the
            # very first matmuls are not stuck behind other transfers
            prefetch_panel(0)
            prefetch_panel(1)
            prefetch_panel(2)
            nc.sync.dma_start(cos_sb[:], cos_d[:])
            nc.sync.dma_start(sin_sb[:], sin_d[:])
            nc.sync.dma_start(mask_sb[:], mask_d[:])
            nc.sync.dma_start(ones_sb[:], ones_d[:])
            # V weights are small enough to keep resident: ONE load, reused
            # by every chunk's V projection.
            wvb = sb.tile([128, KT * KVCOLS], BF16, name="wvb")
            nc.sync.dma_start(wvb[:], wv_d[:])

            def xt_slot(c):
                return c % 2

            def load_xt(c):
                """Four batched DMAs (8 kt-tiles each) bringing chunk c of
                X^T into its rotation slot. (Each DMA instruction costs
                ~0.7us of serial dispatch on the Sync sequencer — batch;
                but keep kt-granularity coarse enough for the first
                matmuls to start before the whole chunk lands.)"""
                sl = xt_slot(c)
                dst = xtb.rearrange("p (kt s t) -> p kt s t", s=2, t=TC)[:, :, sl, :]
                src = xt_d[:, c * TC:(c + 1) * TC].rearrange(
                    "(kt p) t -> p kt t", p=128)
                for g in range(4):
                    nc.sync.dma_start(dst[:, g * 8:(g + 1) * 8, :],
                                      src[:, g * 8:(g + 1) * 8, :])

            def xt_tile(c, kt, lo=0, width=TC):
                sl = xt_slot(c)
                base = kt * 2 * TC + sl * TC + lo
                return xtb[:, base:base + width]

            def rope(dst, pq, t0):
                """dst (bf16 [128, TC]) = rope of pq (fp32 PSUM [128, TC]).
                PSUM drain on the Scalar engine (fast pj-slot recycling),
                half-swap DMAs on the Scalar DGE (ready the moment the
                drain retires), math on DVE. Deep qf/qs rotations so the
                swap dispatches never block the Scalar sequencer on slot
                reuse."""
                qf = sb.tile([128, TC], F32, tag="qf", bufs=3)
                nc.scalar.copy(qf[:], pq[:])
                qs = sb.tile([128, TC], F32, tag="qs", bufs=3)
                nc.scalar.dma_start(qs[0:64, :], qf[64:128, :])
                nc.scalar.dma_start(qs[64:128, :], qf[0:64, :])
                nc.vector.tensor_tensor(qf[:], qf[:], cos_sb[:, t0:t0 + TC], MUL)
                nc.vector.tensor_tensor(qs[:], qs[:], sin_sb[:, t0:t0 + TC], MUL)
                nc.vector.tensor_tensor(dst, qf[:], qs[:], ADD)

            qtb = None

            def proj(c):
                nonlocal qtb
                t0 = c * TC
                qtb = sb.tile([128, HL * TC], BF16, tag="qt", bufs=1, name="qtb")
                for h in range(HL):
                    wb = _WB_PRE.pop(h, None) if c == 0 else None
                    if wb is None:
                        wb = sb.tile([128, KT * 128], BF16, tag="wb", bufs=3,
                                     name="wb")
                        nc.sync.dma_start(wb[:], wq_d[h])
                    pq = ps.tile([128, TC], F32, tag="pj", bufs=2, name="pq")
                    for kt in range(KT):
                        nc.tensor.matmul(
                            pq[:], wb[:, kt * 128:(kt + 1) * 128],
                            xt_tile(c, kt),
                            start=(kt == 0), stop=(kt == KT - 1))
                    rope(qtb[:, h * TC:(h + 1) * TC], pq, t0)
                    if h == 2 and c + 1 < NCHUNK:
                        # next chunk's X^T, after the first panels are in
                        # flight so it doesn't delay them in dispatch order
                        load_xt(c + 1)
                for kv in range(KVL):
                    wb = sb.tile([128, KT * 128], BF16, tag="wb", bufs=3, name="wbk")
                    nc.sync.dma_start(wb[:], wk_d[kv])
                    pk = ps.tile([128, TC], F32, tag="pj", bufs=2, name="pk")
                    for kt in range(KT):
                        nc.tensor.matmul(
                            pk[:], wb[:, kt * 128:(kt + 1) * 128],
                            xt_tile(c, kt),
                            start=(kt == 0), stop=(kt == KT - 1))
                    rope(ktb[:, kv * S + t0:kv * S + t0 + TC], pk, t0)
                # V: lhsT = X^T tiles (stationary), rhs = wv (both kv heads
                # at once, 256-wide) -> V[t, c] accumulated over kt.
                for tt in range(TC // 128):
                    pv = ps.tile([128, KVCOLS], F32, tag="pj", bufs=2, name="pv")
                    for kt in range(KT):
                        lx = xt_tile(c, kt, tt * 128, 128)
                        nc.tensor.matmul(
                            pv[:], lx,
                            wvb[:, kt * KVCOLS:(kt + 1) * KVCOLS],
                            start=(kt == 0), stop=(kt == KT - 1))
                    vt_idx = (t0 // 128) + tt
                    nc.scalar.copy(
                        vb[:, vt_idx * KVCOLS:(vt_idx + 1) * KVCOLS], pv[:])

            attnb = None
            _WOR_PRE = {}

            def prefetch_wor(ocg):
                """ONE batched DMA for the 8 wo rhs slices of output group
                ocg: tile [128, ft*TC + oc]."""
                w = sb.tile([128, HL * TC], BF16, tag="wor", bufs=2,
                            name="wor")
                nc.sync.dma_start(
                    w.rearrange("p (f c) -> p f c", c=TC),
                    wo_d.rearrange("f p c -> p f c")[:, :, ocg * TC:(ocg + 1) * TC])
                _WOR_PRE[ocg] = w
                return w

            def attention(c):
                """Heads processed in pairs sharing a kv head; adjacent in
                qtb, so one [128, 2*TC] score matmul + one fused exp serve
                both. pa/pd accumulate per head; masks alternate DVE/GpSimd."""
                nonlocal attnb
                nkt = 4 * c + 4
                attnb = sb.tile([128, HL * TC], BF16, tag="attn", bufs=1,
                                name="attnb")
                # For chunk 0 (every k-tile diagonal, heavy DVE masking) the
                # denominator rides per-k-tile ones-matmuls on TensorE. For
                # chunks 1-3 the P^T tiles are instead accumulated into
                # per-head running sums (h0-half on DVE, h1-half on GpSimd,
                # both of which have slack) and the denominator costs ONE
                # matmul per head — removing a third of attention's
                # TensorE work.
                use_ptsum = (c > 0)
                for p in range(HL // 2):
                    h0, h1 = 2 * p, 2 * p + 1
                    kv = h0 // (HL // KVL)
                    qpair = qtb[:, h0 * TC:(h0 + 2) * TC]
                    pa0 = ps.tile([128, TC], F32, tag="pa", bufs=2, name="pa0")
                    pa1 = ps.tile([128, TC], F32, tag="pa", bufs=2, name="pa1")
                    pd0 = ps.tile([128, TC], F32, tag="pj", bufs=2, name="pd0")
                    pd1 = ps.tile([128, TC], F32, tag="pj", bufs=2, name="pd1")
                    if use_ptsum:
                        ps0 = sb.tile([128, TC], BF16, tag="ptsum", bufs=2,
                                      name="ps0")
                        ps1 = sb.tile([128, TC], BF16, tag="ptsum", bufs=2,
                                      name="ps1")

                    def emit_pa_pd(kt, pt):
                        st, sp = (kt == 0), (kt == nkt - 1)
                        vt = vb[:, kt * KVCOLS + kv * 128:
                                kt * KVCOLS + (kv + 1) * 128]
                        nc.tensor.matmul(pa0[:], vt, pt[:, :TC],
                                         start=st, stop=sp)
                        nc.tensor.matmul(pa1[:], vt, pt[:, TC:],
                                         start=st, stop=sp)
                        if not use_ptsum:
                            nc.tensor.matmul(pd0[:], ones_sb[:], pt[:, :TC],
                                             start=st, stop=sp)
                            nc.tensor.matmul(pd1[:], ones_sb[:], pt[:, TC:],
                                             start=st, stop=sp)

                    pending = []
                    for kt in range(nkt):
                        kts = ktb[:, kv * S + kt * 128:kv * S + (kt + 1) * 128]
                        sps = ps.tile([128, 2 * TC], F32, tag="s", bufs=2,
                                      name="sps")
                        # matmul out must stay within one PSUM bank: write
                        # the fused tile's halves with two 512-wide matmuls
                        # (same stationary K tile), then ONE fused exp.
                        nc.tensor.matmul(sps[:, :TC], kts,
                                         qpair[:, :TC], start=True, stop=True)
                        nc.tensor.matmul(sps[:, TC:], kts,
                                         qpair[:, TC:], start=True, stop=True)
                        # pa/pd run TWO k-tiles behind the scores: the extra
                        # slack hides exp latency AND the previous pair's
                        # normalize chain (which frees the pa psum slots).
                        if len(pending) >= 2:
                            emit_pa_pd(*pending.pop(0))
                        pt = sb.tile([128, 2 * TC], BF16, tag="pt", bufs=5,
                                     name="pt")
                        nc.scalar.activation(pt[:], sps[:], EXP, scale=SCALE)
                        j = kt - 4 * c
                        if j >= 0:
                            # masks on DVE: GpSimd must stay nearly free so
                            # a collective's completion wait blocks little
                            msk = mask_sb[:, j * 2 * TC:(j + 1) * 2 * TC]
                            nc.vector.tensor_tensor(pt[:], pt[:], msk, MUL)
                        if use_ptsum:
                            if kt == 0:
                                nc.vector.tensor_copy(ps0[:], pt[:, :TC])
                                nc.gpsimd.tensor_copy(ps1[:], pt[:, TC:])
                            else:
                                nc.vector.tensor_tensor(
                                    ps0[:], ps0[:], pt[:, :TC], ADD)
                                nc.gpsimd.tensor_tensor(
                                    ps1[:], ps1[:], pt[:, TC:], ADD)
                        pending.append((kt, pt))
                    while pending:
                        emit_pa_pd(*pending.pop(0))
                    if use_ptsum:
                        nc.tensor.matmul(pd0[:], ones_sb[:], ps0[:],
                                         start=True, stop=True)
                        nc.tensor.matmul(pd1[:], ones_sb[:], ps1[:],
                                         start=True, stop=True)
                    # stage pd through SBUF (ACT copy): frees the pd
                    # psum slot early, and the custom-DVE approx reciprocal
                    # reads SBUF (it is unreliable on PSUM inputs)
                    pdf0 = sb.tile([128, TC], F32, tag="pdf", bufs=2, name="pdf0")
                    nc.scalar.copy(pdf0[:], pd0[:])
                    pdf1 = sb.tile([128, TC], F32, tag="pdf", bufs=2, name="pdf1")
                    nc.scalar.copy(pdf1[:], pd1[:])
                    rc0 = sb.tile([128, TC], F32, tag="rc", bufs=1, name="rc0")
                    nc.vector.reciprocal_approx_fast(rc0[:], pdf0[:])
                    nc.vector.tensor_tensor(
                        attnb[:, h0 * TC:(h0 + 1) * TC], pa0[:], rc0[:], MUL)
                    rc1 = sb.tile([128, TC], F32, tag="rc", bufs=1, name="rc1")
                    nc.vector.reciprocal_approx_fast(rc1[:], pdf1[:])
                    nc.vector.tensor_tensor(
                        attnb[:, h1 * TC:(h1 + 1) * TC], pa1[:], rc1[:], MUL)
                    if p == 2:
                        # hoist first o-proj weight loads under attention
                        prefetch_wor(0)

            def outproj(c):
                """Local partial o_proj -> cci[rank, t, oc], then bf16
                ReduceScatter(add) within the 4-core group. The cco->out_t
                DMAs are deferred to the end of the program so no in-order
                DMA queue mid-stream carries a descriptor that has to wait
                for a collective. The last chunk's RS is split in two
                halves (even / odd output groups) so the first RS overlaps
                the second half of the out-projection matmuls."""
                t0 = c * TC
                ccis = [dr.tile([TP, TC, OC // 2], BF16, tag="cci",
                                bufs=4, name=f"cci{h}") for h in range(2)]
                order = [0, 2, 4, 6, 1, 3, 5, 7]

                pending_out = []

                def emit_rs(half):
                    """Trigger the RS for one half of the output columns.
                    The cco->out DMA rides the GpSimd stream (its wait for
                    the RS blocks only GpSimd, which hosts no other work —
                    a sync-engine DMA here would stall ALL DMA dispatch for
                    the collective's latency), and is DEFERRED past any
                    later RS trigger so back-to-back collectives don't
                    serialize on it."""
                    cco = dr.tile([TC, OC // 2], BF16, tag="cco", bufs=4,
                                  name=f"cco{half}")
                    dst = out_d[t0:t0 + TC,
                                half * (OC // 2):(half + 1) * (OC // 2)]
                    nc.gpsimd.collective_compute(
                        "ReduceScatter", ADD,
                        replica_groups=GROUPS, ins=[ccis[half][:]],
                        outs=[cco[:]])
                    pending_out.append((dst, cco))

                for i, ocg in enumerate(order):
                    wor = _WOR_PRE.pop(ocg, None)
                    if wor is None:
                        wor = prefetch_wor(ocg)
                    if i + 1 < 8:
                        prefetch_wor(order[i + 1])
                    cci = ccis[ocg % 2]
                    ot = sb.tile([128, 4 * TC], BF16, tag="ot", bufs=2,
                                 name="ot")
                    for tt in range(TC // 128):
                        po = ps.tile([128, TC], F32, tag="s", bufs=2, name="po")
                        for ft in range(HL):
                            nc.tensor.matmul(
                                po[:],
                                attnb[:, ft * TC + tt * 128:ft * TC + (tt + 1) * 128],
                                wor[:, ft * TC:(ft + 1) * TC],
                                start=(ft == 0), stop=(ft == HL - 1))
                        nc.scalar.copy(ot[:, tt * TC:(tt + 1) * TC], po[:])
                    nc.sync.dma_start(
                        cci[ocg // 2].rearrange("(tt p) oc -> p tt oc", p=128),
                        ot.rearrange("p (tt oc) -> p tt oc", oc=TC))
                    if i == 3:
                        emit_rs(0)
                emit_rs(1)
                for dst, cco in pending_out:
                    nc.gpsimd.dma_start(dst, cco[:])

            # ---- chunk loop ----
            load_xt(0)
            for c in range(NCHUNK):
                proj(c)
                attention(c)
                outproj(c)

    nc.compile()
    return nc


def _get_nc():
    if "nc" not in _BUILT:
        _BUILT["nc"] = _build()
    return _BUILT["nc"]


def kernel(hidden_states, cos, sin, wq, wk, wv, wo):
    global LAST_RESULT
    nc = _get_nc()

    hidden_states = np.asarray(hidden_states, dtype=np.float32)
    cos = np.asarray(cos, dtype=np.float32)
    sin = np.asarray(sin, dtype=np.float32)
    wq = np.asarray(wq, dtype=np.float32)
    wk = np.asarray(wk, dtype=np.float32)
    wv = np.asarray(wv, dtype=np.float32)
    wo = np.asarray(wo, dtype=np.float32)

    # host-side shard prep (bf16, panel-major so every DMA is contiguous)
    xts = [np.ascontiguousarray(hidden_states[b].T).astype(B16NP)
           for b in range(B)]
    cts = [np.ascontiguousarray(cos[b].T).astype(B16NP) for b in range(B)]
    sin_eff = []
    for b in range(B):
        se = np.ascontiguousarray(sin[b].T).copy()
        se[0:64, :] *= -1.0
        sin_eff.append(se.astype(B16NP))

    # fused pair masks: each diagonal mask duplicated for the head pair
    masks = np.zeros((HD, 8 * TC), dtype=B16NP)
    for j in range(4):
        m = ((np.arange(HD)[:, None] + 128 * j)
             <= np.arange(TC)[None, :]).astype(B16NP)
        masks[:, j * 2 * TC:j * 2 * TC + TC] = m
        masks[:, j * 2 * TC + TC:(j + 1) * 2 * TC] = m
    onesb = np.ones((128, 128), dtype=B16NP)

    in_maps = []
    for core in range(N_CORES):
        b, r = core // TP, core % TP
        wq_s = wq[:, r * QCOLS:(r + 1) * QCOLS]
        wk_s = wk[:, r * KVCOLS:(r + 1) * KVCOLS]
        wv_s = wv[:, r * KVCOLS:(r + 1) * KVCOLS]
        wo_s = wo[r * QCOLS:(r + 1) * QCOLS, :]
        wqp = np.ascontiguousarray(
            wq_s.reshape(KT, 128, HL, 128).transpose(2, 1, 0, 3)
            .reshape(HL, 128, KT * 128)).astype(B16NP)
        wkp = np.ascontiguousarray(
            wk_s.reshape(KT, 128, KVL, 128).transpose(2, 1, 0, 3)
            .reshape(KVL, 128, KT * 128)).astype(B16NP)
        wvp = np.ascontiguousarray(
            wv_s.reshape(KT, 128, KVCOLS).transpose(1, 0, 2)
            .reshape(128, KT * KVCOLS)).astype(B16NP)
        wop = np.ascontiguousarray(
            wo_s.reshape(HL, 128, H)).astype(B16NP)
        in_maps.append({
            "xt": xts[b],
            "cos_t": cts[b],
            "sin_t": sin_eff[b],
            "wqp": wqp,
            "wkp": wkp,
            "wvp": wvp,
            "wop": wop,
            "masks": masks,
            "onesb": onesb,
        })

    res = run_bass_kernel_spmd(nc, in_maps, core_ids=list(range(N_CORES)))
    LAST_RESULT = res

    out = np.empty((B, S, H), dtype=np.float32)
    for core in range(N_CORES):
        b, r = core // TP, core % TP
        out[b, :, r * OC:(r + 1) * OC] = res.results[core]["out_t"].astype(
            np.float32)
    return out


# revision 41
# speedup vs baseline: 1.0153x; 1.0046x over previous
"""Llama GQA attention (B=2, S=2048, H=4096, 32 q heads / 8 kv heads, HD=128)
on 8 Trainium2 NeuronCores.

Sharding: DP=2 over batch x TP=4 over heads.
  core c: batch b = c // 4, tp rank r = c % 4
  - owns q heads [8r, 8r+8), kv heads [2r, 2r+2)
  - computes attention for its heads over its batch
  - o_proj: LOCAL partial over its 1024 attn features for ALL 4096 out
    cols (wo sharded by ROWS), then bf16 ReduceScatter(add) within each
    4-core batch group -> each core holds out cols [1024r, 1024(r+1)).

All inputs are pre-cast to bf16 on the host (weights in panel-major
layouts so every DMA is contiguous), so the device pipeline has zero
dtype-conversion work and no DRAM scratch round-trips.

On-chip layout is fully "transposed" ([feature, token]):
  QT/KT: [d, t] (weight tiles stationary, X^T moving)
  V:     [t, d] (X^T tiles stationary, wv moving)
  S^T[k, q] = (KT tile).T @ QT          (contraction d on partitions)
  P^T = exp(scale * S^T)                (ScalarE, fp32 PSUM -> bf16 SBUF)
  attn^T[d, q] += (V tile).T @ P^T      (contraction k-tokens on partitions)
  out[t, oc]  += (attnb tile).T @ wo    (contraction d on partitions)
Causal masking: only lower-triangle k-tiles are computed; diagonal
128x512 tiles use one of 4 static 0/1 masks (multiplied into P^T on DVE).
Softmax skips max-subtraction (scores are O(7), exp fits fp32).

Denominators: P^T tiles are accumulated on the Pool engine into a
per-head running sum (bf16), then ONE ones-matmul per (head, chunk)
produces the denominator — instead of one matmul per k-tile.

Attention processes heads in PAIRS sharing the same kv head, software-
pipelined so the exp->mask->pa chain of one head hides under the other
head's matmuls:
  per kt: [score(h0), score(h1), pa(h0, kt-1), pa(h1, kt-1)]
X^T for chunk c+1 is DMA'd (pure bf16 copy, no engine work) into a
2-slot rotation at the start of proj(c).
"""

import os
import sys

for _p in ("/opt/trn_rl_repo",):
    if _p not in sys.path:
        sys.path.append(_p)

import numpy as np
import ml_dtypes

import concourse.bacc as bacc
import concourse.mybir as mybir
import concourse.tile as tile
from concourse.bass_utils import run_bass_kernel_spmd

F32 = mybir.dt.float32
BF16 = mybir.dt.bfloat16
B16NP = ml_dtypes.bfloat16

B, S, H = 2, 2048, 4096
NH, NKV, HD = 32, 8, 128
N_CORES = 8
TP = 4
GROUPS = [[0, 1, 2, 3], [4, 5, 6, 7]]

HL = NH // TP          # 8 local q heads
KVL = NKV // TP        # 2 local kv heads
QCOLS = HL * HD        # 1024 local q cols
KVCOLS = KVL * HD      # 256 local kv cols
OC = H // TP           # 1024 out cols per core after ReduceScatter

TC = 512               # token chunk (= one attention q-block)
NCHUNK = S // TC       # 4
KT = H // 128          # 32 contraction tiles for the projections
SCALE = float(HD ** -0.5)

LAST_RESULT = None
_BUILT = {}

EXP = mybir.ActivationFunctionType.Exp
MUL = mybir.AluOpType.mult
ADD = mybir.AluOpType.add


def _build():
    nc = bacc.Bacc("TRN2", debug=False, num_devices=N_CORES)

    xt_d = nc.dram_tensor("xt", [H, S], BF16, kind="ExternalInput").ap()
    cos_d = nc.dram_tensor("cos_t", [HD, S], BF16, kind="ExternalInput").ap()
    sin_d = nc.dram_tensor("sin_t", [HD, S], BF16, kind="ExternalInput").ap()
    wq_d = nc.dram_tensor("wqp", [HL, 128, KT * 128], BF16, kind="ExternalInput").ap()
    wk_d = nc.dram_tensor("wkp", [KVL, 128, KT * 128], BF16, kind="ExternalInput").ap()
    wv_d = nc.dram_tensor("wvp", [128, KT * KVCOLS], BF16, kind="ExternalInput").ap()
    wo_d = nc.dram_tensor("wop", [HL, 128, H], BF16, kind="ExternalInput").ap()
    mask_d = nc.dram_tensor("masks", [HD, 8 * TC], BF16, kind="ExternalInput").ap()
    ones_d = nc.dram_tensor("onesb", [128, 128], BF16, kind="ExternalInput").ap()
    out_d = nc.dram_tensor("out_t", [S, OC], BF16, kind="ExternalOutput").ap()

    with tile.TileContext(nc) as tc:
        with tc.tile_pool(name="sb", bufs=1) as sb, \
             tc.tile_pool(name="ps", bufs=1, space="PSUM") as ps, \
             tc.tile_pool(name="dr", bufs=1, space="DRAM") as dr:

            # ---- persistent tiles ----
            cos_sb = sb.tile([HD, S], BF16)
            sin_sb = sb.tile([HD, S], BF16)
            mask_sb = sb.tile([HD, 8 * TC], BF16)
            ones_sb = sb.tile([128, 128], BF16)
            ktb = sb.tile([128, KVL * S], BF16)             # roped K^T
            vb = sb.tile([128, (S // 128) * KVCOLS], BF16)  # V, [t, tt*256 + d]
            # X^T bf16, 2-chunk rotation: [128, kt*(2*TC) + slot*TC + t]
            xtb = sb.tile([128, KT * 2 * TC], BF16)

            _WB_PRE = {}

            def prefetch_panel(h):
                wb = sb.tile([128, KT * 128], BF16, tag="wb", bufs=3,
                             name="wb")
                nc.sync.dma_start(wb[:], wq_d[h])
                _WB_PRE[h] = wb
                return wb

            # first two q panels dispatched before anything else so the
            # very first matmuls are not stuck behind other transfers
            prefetch_panel(0)
            prefetch_panel(1)
            prefetch_panel(2)
            nc.sync.dma_start(cos_sb[:], cos_d[:])
            nc.sync.dma_start(sin_sb[:], sin_d[:])
            nc.sync.dma_start(mask_sb[:], mask_d[:])
            nc.sync.dma_start(ones_sb[:], ones_d[:])
            # V weights are small enough to keep resident: ONE load, reused
            # by every chunk's V projection.
            wvb = sb.tile([128, KT * KVCOLS], BF16, name="wvb")
            nc.sync.dma_start(wvb[:], wv_d[:])

            def xt_slot(c):
                return c % 2

            def load_xt(c):
                """Four batched DMAs (8 kt-tiles each) bringing chunk c of
                X^T into its rotation slot. (Each DMA instruction costs
                ~0.7us of serial dispatch on the Sync sequencer — batch;
                but keep kt-granularity coarse enough for the first
                matmuls to start before the whole chunk lands.)"""
                sl = xt_slot(c)
                dst = xtb.rearrange("p (kt s t) -> p kt s t", s=2, t=TC)[:, :, sl, :]
                src = xt_d[:, c * TC:(c + 1) * TC].rearrange(
                    "(kt p) t -> p kt t", p=128)
                for g in range(4):
                    nc.sync.dma_start(dst[:, g * 8:(g + 1) * 8, :],
                                      src[:, g * 8:(g + 1) * 8, :])

            def xt_tile(c, kt, lo=0, width=TC):
                sl = xt_slot(c)
                base = kt * 2 * TC + sl * TC + lo
                return xtb[:, base:base + width]

            def rope(dst, pq, t0):
                """dst (bf16 [128, TC]) = rope of pq (fp32 PSUM [128, TC]).
                PSUM drain on the Scalar engine (fast pj-slot recycling),
                half-swap DMAs on the Scalar DGE (ready the moment the
                drain retires), math on DVE. Deep qf/qs rotations so the
                swap dispatches never block the Scalar sequencer on slot
                reuse."""
                qf = sb.tile([128, TC], F32, tag="qf", bufs=3)
                nc.scalar.copy(qf[:], pq[:])
                qs = sb.tile([128, TC], F32, tag="qs", bufs=3)
                nc.scalar.dma_start(qs[0:64, :], qf[64:128, :])
                nc.scalar.dma_start(qs[64:128, :], qf[0:64, :])
                nc.vector.tensor_tensor(qf[:], qf[:], cos_sb[:, t0:t0 + TC], MUL)
                nc.vector.tensor_tensor(qs[:], qs[:], sin_sb[:, t0:t0 + TC], MUL)
                nc.vector.tensor_tensor(dst, qf[:], qs[:], ADD)

            qtb = None

            def proj(c):
                nonlocal qtb
                t0 = c * TC
                qtb = sb.tile([128, HL * TC], BF16, tag="qt", bufs=1, name="qtb")
                for h in range(HL):
                    wb = _WB_PRE.pop(h, None) if c == 0 else None
                    if wb is None:
                        wb = sb.tile([128, KT * 128], BF16, tag="wb", bufs=3,
                                     name="wb")
                        nc.sync.dma_start(wb[:], wq_d[h])
                    pq = ps.tile([128, TC], F32, tag="pj", bufs=2, name="pq")
                    for kt in range(KT):
                        nc.tensor.matmul(
                            pq[:], wb[:, kt * 128:(kt + 1) * 128],
                            xt_tile(c, kt),
                            start=(kt == 0), stop=(kt == KT - 1))
                    rope(qtb[:, h * TC:(h + 1) * TC], pq, t0)
                    if h == 2 and c + 1 < NCHUNK:
                        # next chunk's X^T, after the first panels are in
                        # flight so it doesn't delay them in dispatch order
                        load_xt(c + 1)
                for kv in range(KVL):
                    wb = sb.tile([128, KT * 128], BF16, tag="wb", bufs=3, name="wbk")
                    nc.sync.dma_start(wb[:], wk_d[kv])
                    pk = ps.tile([128, TC], F32, tag="pj", bufs=2, name="pk")
                    for kt in range(KT):
                        nc.tensor.matmul(
                            pk[:], wb[:, kt * 128:(kt + 1) * 128],
                            xt_tile(c, kt),
                            start=(kt == 0), stop=(kt == KT - 1))
                    rope(ktb[:, kv * S + t0:kv * S + t0 + TC], pk, t0)
                # V: lhsT = X^T tiles (stationary), rhs = wv (both kv heads
                # at once, 256-wide) -> V[t, c] accumulated over kt.
                for tt in range(TC // 128):
                    pv = ps.tile([128, KVCOLS], F32, tag="pj", bufs=2, name="pv")
                    for kt in range(KT):
                        lx = xt_tile(c, kt, tt * 128, 128)
                        nc.tensor.matmul(
                            pv[:], lx,
                            wvb[:, kt * KVCOLS:(kt + 1) * KVCOLS],
                            start=(kt == 0), stop=(kt == KT - 1))
                    vt_idx = (t0 // 128) + tt
                    nc.scalar.copy(
                        vb[:, vt_idx * KVCOLS:(vt_idx + 1) * KVCOLS], pv[:])

            attnb = None
            _WOR_PRE = {}

            def prefetch_wor(ocg):
                """ONE batched DMA for the 8 wo rhs slices of output group
                ocg: tile [128, ft*TC + oc]."""
                w = sb.tile([128, HL * TC], BF16, tag="wor", bufs=2,
                            name="wor")
                nc.sync.dma_start(
                    w.rearrange("p (f c) -> p f c", c=TC),
                    wo_d.rearrange("f p c -> p f c")[:, :, ocg * TC:(ocg + 1) * TC])
                _WOR_PRE[ocg] = w
                return w

            def attention(c):
                """Heads processed in pairs sharing a kv head; adjacent in
                qtb, so one [128, 2*TC] score matmul + one fused exp serve
                both. pa/pd accumulate per head; masks alternate DVE/GpSimd."""
                nonlocal attnb
                nkt = 4 * c + 4
                attnb = sb.tile([128, HL * TC], BF16, tag="attn", bufs=1,
                                name="attnb")
                # For chunk 0 (every k-tile diagonal, heavy DVE masking) the
                # denominator rides per-k-tile ones-matmuls on TensorE. For
                # chunks 1-3 the P^T tiles are instead accumulated into
                # per-head running sums (h0-half on DVE, h1-half on GpSimd,
                # both of which have slack) and the denominator costs ONE
                # matmul per head — removing a third of attention's
                # TensorE work.
                use_ptsum = (c > 0)
                for p in range(HL // 2):
                    h0, h1 = 2 * p, 2 * p + 1
                    kv = h0 // (HL // KVL)
                    qpair = qtb[:, h0 * TC:(h0 + 2) * TC]
                    pa0 = ps.tile([128, TC], F32, tag="pa", bufs=2, name="pa0")
                    pa1 = ps.tile([128, TC], F32, tag="pa", bufs=2, name="pa1")
                    pd0 = ps.tile([128, TC], F32, tag="pj", bufs=2, name="pd0")
                    pd1 = ps.tile([128, TC], F32, tag="pj", bufs=2, name="pd1")
                    if use_ptsum:
                        ps0 = sb.tile([128, TC], BF16, tag="ptsum", bufs=2,
                                      name="ps0")
                        ps1 = sb.tile([128, TC], BF16, tag="ptsum", bufs=2,
                                      name="ps1")

                    def emit_pa_pd(kt, pt):
                        st, sp = (kt == 0), (kt == nkt - 1)
                        vt = vb[:, kt * KVCOLS + kv * 128:
                                kt * KVCOLS + (kv + 1) * 128]
                        nc.tensor.matmul(pa0[:], vt, pt[:, :TC],
                                         start=st, stop=sp)
                        nc.tensor.matmul(pa1[:], vt, pt[:, TC:],
                                         start=st, stop=sp)
                        if not use_ptsum:
                            nc.tensor.matmul(pd0[:], ones_sb[:], pt[:, :TC],
                                             start=st, stop=sp)
                            nc.tensor.matmul(pd1[:], ones_sb[:], pt[:, TC:],
                                             start=st, stop=sp)

                    pending = []
                    for kt in range(nkt):
                        kts = ktb[:, kv * S + kt * 128:kv * S + (kt + 1) * 128]
                        sps = ps.tile([128, 2 * TC], F32, tag="s", bufs=2,
                                      name="sps")
                        # matmul out must stay within one PSUM bank: write
                        # the fused tile's halves with two 512-wide matmuls
                        # (same stationary K tile), then ONE fused exp.
                        nc.tensor.matmul(sps[:, :TC], kts,
                                         qpair[:, :TC], start=True, stop=True)
                        nc.tensor.matmul(sps[:, TC:], kts,
                                         qpair[:, TC:], start=True, stop=True)
                        # pa/pd run TWO k-tiles behind the scores: the extra
                        # slack hides exp latency AND the previous pair's
                        # normalize chain (which frees the pa psum slots).
                        if len(pending) >= 2:
                            emit_pa_pd(*pending.pop(0))
                        pt = sb.tile([128, 2 * TC], BF16, tag="pt", bufs=5,
                                     name="pt")
                        nc.scalar.activation(pt[:], sps[:], EXP, scale=SCALE)
                        j = kt - 4 * c
                        if j >= 0:
                            # masks on DVE: GpSimd must stay nearly free so
                            # a collective's completion wait blocks little
                            msk = mask_sb[:, j * 2 * TC:(j + 1) * 2 * TC]
                            nc.vector.tensor_tensor(pt[:], pt[:], msk, MUL)
                        if use_ptsum:
                            if kt == 0:
                                nc.vector.tensor_copy(ps0[:], pt[:, :TC])
                                nc.gpsimd.tensor_copy(ps1[:], pt[:, TC:])
                            else:
                                nc.vector.tensor_tensor(
                                    ps0[:], ps0[:], pt[:, :TC], ADD)
                                nc.gpsimd.tensor_tensor(
                                    ps1[:], ps1[:], pt[:, TC:], ADD)
                        pending.append((kt, pt))
                    while pending:
                        emit_pa_pd(*pending.pop(0))
                    if use_ptsum:
                        nc.tensor.matmul(pd0[:], ones_sb[:], ps0[:],
                                         start=True, stop=True)
                        nc.tensor.matmul(pd1[:], ones_sb[:], ps1[:],
                                         start=True, stop=True)
                    # stage pd through SBUF (ACT copy): frees the pd
                    # psum slot early, and the custom-DVE approx reciprocal
                    # reads SBUF (it is unreliable on PSUM inputs)
                    pdf0 = sb.tile([128, TC], F32, tag="pdf", bufs=2, name="pdf0")
                    nc.scalar.copy(pdf0[:], pd0[:])
                    pdf1 = sb.tile([128, TC], F32, tag="pdf", bufs=2, name="pdf1")
                    nc.scalar.copy(pdf1[:], pd1[:])
                    rc0 = sb.tile([128, TC], F32, tag="rc", bufs=1, name="rc0")
                    nc.vector.reciprocal_approx_fast(rc0[:], pdf0[:])
                    nc.vector.tensor_tensor(
                        attnb[:, h0 * TC:(h0 + 1) * TC], pa0[:], rc0[:], MUL)
                    rc1 = sb.tile([128, TC], F32, tag="rc", bufs=1, name="rc1")
                    nc.vector.reciprocal_approx_fast(rc1[:], pdf1[:])
                    nc.vector.tensor_tensor(
                        attnb[:, h1 * TC:(h1 + 1) * TC], pa1[:], rc1[:], MUL)
                    if p == 2:
                        # hoist first o-proj weight loads under attention
                        prefetch_wor(0)

            def outproj(c):
                """Local partial o_proj -> cci[rank, t, oc], then bf16
                ReduceScatter(add) within the 4-core group. The cco->out_t
                DMAs are deferred to the end of the program so no in-order
                DMA queue mid-stream carries a descriptor that has to wait
                for a collective. The last chunk's RS is split in two
                halves (even / odd output groups) so the first RS overlaps
                the second half of the out-projection matmuls."""
                t0 = c * TC
                ccis = [dr.tile([TP, TC, OC // 2], BF16, tag="cci",
                                bufs=4, name=f"cci{h}") for h in range(2)]
                order = [0, 2, 4, 6, 1, 3, 5, 7]

                pending_out = []

                def emit_rs(half):
                    """Trigger the RS for one half of the output columns.
                    The cco->out DMA rides the GpSimd stream (its wait for
                    the RS blocks only GpSimd, which hosts no other work —
                    a sync-engine DMA here would stall ALL DMA dispatch for
                    the collective's latency), and is DEFERRED past any
                    later RS trigger so back-to-back collectives don't
                    serialize on it."""
                    cco = dr.tile([TC, OC // 2], BF16, tag="cco", bufs=4,
                                  name=f"cco{half}")
                    dst = out_d[t0:t0 + TC,
                                half * (OC // 2):(half + 1) * (OC // 2)]
                    nc.gpsimd.collective_compute(
                        "ReduceScatter", ADD,
                        replica_groups=GROUPS, ins=[ccis[half][:]],
                        outs=[cco[:]])
                    pending_out.append((dst, cco))

                for i, ocg in enumerate(order):
                    wor = _WOR_PRE.pop(ocg, None)
                    if wor is None:
                        wor = prefetch_wor(ocg)
                    if i + 1 < 8:
                        prefetch_wor(order[i + 1])
                    cci = ccis[ocg % 2]
                    ot = sb.tile([128, 4 * TC], BF16, tag="ot", bufs=2,
                                 name="ot")
                    for tt in range(TC // 128):
                        po = ps.tile([128, TC], F32, tag="s", bufs=2, name="po")
                        for ft in range(HL):
                            nc.tensor.matmul(
                                po[:],
                                attnb[:, ft * TC + tt * 128:ft * TC + (tt + 1) * 128],
                                wor[:, ft * TC:(ft + 1) * TC],
                                start=(ft == 0), stop=(ft == HL - 1))
                        # DVE is idle during outproj; ACT must stay clear
                        # so the next attention's exps start on time
                        nc.vector.tensor_copy(ot[:, tt * TC:(tt + 1) * TC], po[:])
                    nc.sync.dma_start(
                        cci[ocg // 2].rearrange("(tt p) oc -> p tt oc", p=128),
                        ot.rearrange("p (tt oc) -> p tt oc", oc=TC))
                    if i == 3:
                        emit_rs(0)
                emit_rs(1)
                for dst, cco in pending_out:
                    nc.gpsimd.dma_start(dst, cco[:])

            # ---- chunk loop ----
            load_xt(0)
            for c in range(NCHUNK):
                proj(c)
                attention(c)
                outproj(c)

    nc.compile()
    return nc


def _get_nc():
    if "nc" not in _BUILT:
        _BUILT["nc"] = _build()
    return _BUILT["nc"]


def kernel(hidden_states, cos, sin, wq, wk, wv, wo):
    global LAST_RESULT
    nc = _get_nc()

    hidden_states = np.asarray(hidden_states, dtype=np.float32)
    cos = np.asarray(cos, dtype=np.float32)
    sin = np.asarray(sin, dtype=np.float32)
    wq = np.asarray(wq, dtype=np.float32)
    wk = np.asarray(wk, dtype=np.float32)
    wv = np.asarray(wv, dtype=np.float32)
    wo = np.asarray(wo, dtype=np.float32)

    # host-side shard prep (bf16, panel-major so every DMA is contiguous)
    xts = [np.ascontiguousarray(hidden_states[b].T).astype(B16NP)
           for b in range(B)]
    cts = [np.ascontiguousarray(cos[b].T).astype(B16NP) for b in range(B)]
    sin_eff = []
    for b in range(B):
        se = np.ascontiguousarray(sin[b].T).copy()
        se[0:64, :] *= -1.0
        sin_eff.append(se.astype(B16NP))

    # fused pair masks: each diagonal mask duplicated for the head pair
    masks = np.zeros((HD, 8 * TC), dtype=B16NP)
    for j in range(4):
        m = ((np.arange(HD)[:, None] + 128 * j)
             <= np.arange(TC)[None, :]).astype(B16NP)
        masks[:, j * 2 * TC:j * 2 * TC + TC] = m
        masks[:, j * 2 * TC + TC:(j + 1) * 2 * TC] = m
    onesb = np.ones((128, 128), dtype=B16NP)

    in_maps = []
    for core in range(N_CORES):
        b, r = core // TP, core % TP
        wq_s = wq[:, r * QCOLS:(r + 1) * QCOLS]
        wk_s = wk[:, r * KVCOLS:(r + 1) * KVCOLS]
        wv_s = wv[:, r * KVCOLS:(r + 1) * KVCOLS]
        wo_s = wo[r * QCOLS:(r + 1) * QCOLS, :]
        wqp = np.ascontiguousarray(
            wq_s.reshape(KT, 128, HL, 128).transpose(2, 1, 0, 3)
            .reshape(HL, 128, KT * 128)).astype(B16NP)
        wkp = np.ascontiguousarray(
            wk_s.reshape(KT, 128, KVL, 128).transpose(2, 1, 0, 3)
            .reshape(KVL, 128, KT * 128)).astype(B16NP)
        wvp = np.ascontiguousarray(
            wv_s.reshape(KT, 128, KVCOLS).transpose(1, 0, 2)
            .reshape(128, KT * KVCOLS)).astype(B16NP)
        wop = np.ascontiguousarray(
            wo_s.reshape(HL, 128, H)).astype(B16NP)
        in_maps.append({
            "xt": xts[b],
            "cos_t": cts[b],
            "sin_t": sin_eff[b],
            "wqp": wqp,
            "wkp": wkp,
            "wvp": wvp,
            "wop": wop,
            "masks": masks,
            "onesb": onesb,
        })

    res = run_bass_kernel_spmd(nc, in_maps, core_ids=list(range(N_CORES)))
    LAST_RESULT = res

    out = np.empty((B, S, H), dtype=np.float32)
    for core in range(N_CORES):
        b, r = core // TP, core % TP
        out[b, :, r * OC:(r + 1) * OC] = res.results[core]["out_t"].astype(
            np.float32)
    return out
